# revision 2
# baseline (speedup 1.0000x reference)
"""Inverse discrete Hough transform on 8 Trainium2 NeuronCores.

out[n, c, y, x] = sum_a acc[n, c, a, r(a, y, x)],
r(a, y, x) = round(x' cos_a + y' sin_a) + R/2  (static index table).

Strategy: pixel-shard (each core owns 32 output rows, all 256 (n,c)
channels so the f32r matmul free dim is 256). Per 16x8 pixel tile, the
per-angle rho index ranges ("bands", ~17 rho values each) of all 180
angles are concatenated into K=128-row passes. Each pass is one PE
matmul: stationary = static one-hot [128 rows x 128 px] mapping each
(angle, rho) row to the pixels that gather it, moving = the matching
acc rows [128 x 256 ch] (host-assembled permutation of the input),
accumulating all passes into one PSUM tile [128 px, 256 ch]. The PE
does ~#angles-per-pass * 128 useful gather-adds per cycle.

Host does only layout work (transpose + row gather); all arithmetic
(the 3.0G gathered adds) runs on the NeuronCores.
"""
import sys, os

sys.path.insert(0, "/opt/trn_rl_repo")
import numpy as np
import ml_dtypes

from concourse import bass, tile
from concourse.bass_utils import run_bass_kernel_spmd
import concourse.mybir as mybir

# ---------------- problem constants (hardcoded) ----------------
OUT_H = 256
OUT_W = 256
NUMANGLE = 180
NUMRHO = 400
N_B, C_CH = 4, 64
NCH = N_B * C_CH  # 256 channels
N_CORES = 8
ROWS_PER_CORE = OUT_H // N_CORES  # 32 y-rows per core
TH, TW = 16, 8  # pixel tile: 16 y x 8 x = 128 px
TILES_Y = ROWS_PER_CORE // TH  # 2
TILES_X = OUT_W // TW  # 32
TILES = TILES_Y * TILES_X  # 64

f32 = mybir.dt.float32
f32r = mybir.dt.float32r
bf16 = mybir.dt.bfloat16

_MAX_INSTR_WAITS = 1


def _split_excess_waits(nc):
    """walrus's TRN2 codegen allows only one sync-wait command on several
    instruction structs (NoOp/Drain CTRL_NO, self-loading f32r Matmult
    S3_LW). Move excess waits onto injected same-engine NoOps placed just
    before the over-subscribed instruction (same-engine program order
    preserves semantics)."""
    n = 0
    for fn in nc.m.functions:
        for bb in fn.blocks:
            out = []
            changed = False
            for inst in bb.instructions:
                si = inst.sync_info
                waits = list(si.on_wait) if si and si.on_wait else []
                if len(waits) > _MAX_INSTR_WAITS:
                    for w in waits[_MAX_INSTR_WAITS:]:
                        nop = mybir.InstNoOp(
                            name=f"waitsplit-{n}-{inst.name}", ins=[], outs=[]
                        )
                        n += 1
                        nop.engine = inst.engine
                        nop.sync_info = mybir.SyncInfo(on_wait=[w], on_update=[])
                        out.append(nop)
                    inst.sync_info = mybir.SyncInfo(
                        on_wait=waits[:_MAX_INSTR_WAITS],
                        on_update=list(si.on_update or []),
                    )
                    changed = True
                out.append(inst)
            if changed:
                bb.instructions = out
    return n


def _install_ntff_hook():
    """Recreate antenv.axon_hooks (trimmed from this image) so
    run_bass_kernel_spmd(trace=True) can capture NTFF profiles."""
    import types
    import antenv

    if hasattr(antenv, "axon_hooks"):
        return
    try:
        from trn_agent_boot.trn_boot import _ntff_profile_via_ctypes
    except ImportError:
        return
    hook = _ntff_profile_via_ctypes("/opt/axon/libaxon_pjrt.so")
    mod = types.ModuleType("antenv.axon_hooks")
    mod.get_axon_ntff_profile_hook = lambda: hook
    mod.set_axon_ntff_profile_hook = lambda h: None
    sys.modules["antenv.axon_hooks"] = mod
    antenv.axon_hooks = mod


_install_ntff_hook()


# ---------------- static index tables ----------------
def _rho_index_table():
    """Mirror of the reference's jnp fp32 math (run through jax so the
    rounding matches the harness's reference bit-for-bit)."""
    try:
        import jax
        import jax.numpy as jnp

        with jax.default_device(jax.devices("cpu")[0]):
            angles = jnp.arange(NUMANGLE, dtype=jnp.float32) * (np.pi / NUMANGLE)
            cos_t = jnp.cos(angles)
            sin_t = jnp.sin(angles)
            xs = (jnp.arange(OUT_W) - OUT_W // 2).astype(jnp.float32)
            ys = (jnp.arange(OUT_H) - OUT_H // 2).astype(jnp.float32)
            r = jnp.round(
                xs[None, None, :] * cos_t[:, None, None]
                + ys[None, :, None] * sin_t[:, None, None]
            ).astype(jnp.int32) + NUMRHO // 2
            valid = (r >= 0) & (r < NUMRHO)
            r = jnp.clip(r, 0, NUMRHO - 1)
            return np.asarray(r), np.asarray(valid)
    except Exception:
        angles = (np.arange(NUMANGLE, dtype=np.float32) * np.float32(np.pi / NUMANGLE)).astype(np.float32)
        cos_t = np.cos(angles).astype(np.float32)
        sin_t = np.sin(angles).astype(np.float32)
        xs = (np.arange(OUT_W) - OUT_W // 2).astype(np.float32)
        ys = (np.arange(OUT_H) - OUT_H // 2).astype(np.float32)
        z = (
            xs[None, None, :] * cos_t[:, None, None]
            + ys[None, :, None] * sin_t[:, None, None]
        )
        r = np.round(z).astype(np.int32) + NUMRHO // 2
        valid = (r >= 0) & (r < NUMRHO)
        r = np.clip(r, 0, NUMRHO - 1)
        return r, valid


_STATIC = {}


def _build_static():
    """Per-core ROWIDX (moving-operand row gather) and one-hot tables."""
    if _STATIC:
        return _STATIC
    r_idx, valid = _rho_index_table()  # [A, H, W]

    per_core_rows = []  # rows arrays [TILES, P, 128] of flat accT row ids
    per_core_oh = []  # one-hot [TILES, 128, P, 128] bf16
    tiles_meta = []  # per core: list of (tile_r [A,128], valid_t [A,128])

    # pass 1: compute band row lists per (core, tile) and global P
    all_rows = []  # (core, tile) -> np [L, 2]
    for core in range(N_CORES):
        y0 = core * ROWS_PER_CORE
        core_rows = []
        core_meta = []
        for t in range(TILES):
            ty, tx = divmod(t, TILES_X)
            ys_ = y0 + ty * TH
            xs_ = tx * TW
            tr = r_idx[:, ys_ : ys_ + TH, xs_ : xs_ + TW].reshape(NUMANGLE, TH * TW)
            tv = valid[:, ys_ : ys_ + TH, xs_ : xs_ + TW].reshape(NUMANGLE, TH * TW)
            core_meta.append((tr, tv))
            lo = tr.min(axis=1)
            hi = tr.max(axis=1)
            widths = hi - lo + 1
            rows = np.zeros((widths.sum(), 2), np.int32)
            pos = 0
            for a in range(NUMANGLE):
                w = widths[a]
                rows[pos : pos + w, 0] = a
                rows[pos : pos + w, 1] = np.arange(lo[a], hi[a] + 1)
                pos += w
            core_rows.append(rows)
        all_rows.append(core_rows)
        tiles_meta.append(core_meta)

    P = max(
        (len(r) + 127) // 128 for core_rows in all_rows for r in core_rows
    )
    _STATIC["P"] = P

    for core in range(N_CORES):
        rowidx = np.zeros((TILES, P * 128), np.int64)
        oh = np.zeros((TILES, P * 128, 128), np.float32)
        for t in range(TILES):
            rows = all_rows[core][t]
            tr, tv = tiles_meta[core][t]
            L = len(rows)
            a_arr = rows[:, 0]
            rho_arr = rows[:, 1]
            # one-hot: row k selects pixels px with r(a_k, px) == rho_k
            oh[t, :L] = (tr[a_arr] == rho_arr[:, None]) & tv[a_arr]
            rowidx[t, :L] = a_arr.astype(np.int64) * NUMRHO + rho_arr
            # padding rows index (0, 0); rho=0 is gathered by no pixel
            # (all r values lie in [18, 382]) so their one-hot rows are 0.
        # device layouts: MOV [128k, P, 256] -> idx [TILES, 128, P]
        rowidx = rowidx.reshape(TILES, P, 128).transpose(0, 2, 1)
        oh_d = np.ascontiguousarray(
            oh.reshape(TILES, P, 128, 128).transpose(0, 2, 1, 3)
        ).astype(ml_dtypes.bfloat16)
        per_core_rows.append(np.ascontiguousarray(rowidx))
        per_core_oh.append(oh_d)

    _STATIC["rowidx"] = per_core_rows
    _STATIC["oh"] = per_core_oh
    return _STATIC


# ---------------- device program ----------------
_PROGRAM = {}


def _build_program(P):
    if "nc" in _PROGRAM:
        return _PROGRAM["nc"]
    nc = bass.Bass()
    mov_dram = nc.declare_dram_parameter("mov", [TILES, 128, P, NCH], f32r, isOutput=False)
    oh_dram = nc.declare_dram_parameter("oh", [TILES, 128, P, 128], bf16, isOutput=False)
    out_dram = nc.declare_dram_parameter("out", [TILES, 128, NCH], f32, isOutput=True)

    with tile.TileContext(nc) as tc:
        with (
            tc.tile_pool(name="mov", bufs=2) as movp,
            tc.tile_pool(name="oh", bufs=2) as ohp,
            tc.tile_pool(name="out", bufs=2) as outp,
            tc.tile_pool(name="psum", bufs=2, space="PSUM") as psump,
        ):
            for t in range(TILES):
                mov_sb = movp.tile([128, P, NCH], f32r)
                nc.sync.dma_start(mov_sb[:], mov_dram[t])
                oh_sb = ohp.tile([128, P, 128], f32r)
                nc.gpsimd.dma_start(oh_sb[:], oh_dram[t])  # bf16 -> f32r cast
                acc_ps = psump.tile([128, NCH], f32)
                for p in range(P):
                    nc.tensor.matmul(
                        acc_ps[:],
                        oh_sb[:, p, :],
                        mov_sb[:, p, :],
                        start=(p == 0),
                        stop=(p == P - 1),
                    )
                y = outp.tile([128, NCH], f32)
                nc.vector.tensor_copy(y[:], acc_ps[:])
                nc.sync.dma_start(out_dram[t], y[:])

    _split_excess_waits(nc)
    _PROGRAM["nc"] = nc
    return nc


# ---------------- entry point ----------------
def _run(accumulator: np.ndarray, trace: bool = False):
    st = _build_static()
    P = st["P"]
    nc = _build_program(P)

    accT = np.ascontiguousarray(
        accumulator.transpose(2, 3, 0, 1)
    ).reshape(NUMANGLE * NUMRHO, NCH)

    in_maps = []
    for core in range(N_CORES):
        mov = accT[st["rowidx"][core]]  # [TILES, 128, P, 256] f32
        in_maps.append({"mov": mov, "oh": st["oh"][core]})

    res = run_bass_kernel_spmd(
        nc, in_maps, list(range(N_CORES)), trace=trace
    )

    # reassemble: out_c [TILES, 128, 256] -> [256ch, 32y, 256x]
    parts = []
    for core in range(N_CORES):
        oc = res.results[core]["out"]  # [64, 128, 256]
        oc = oc.reshape(TILES_Y, TILES_X, TH, TW, NCH).transpose(4, 0, 2, 1, 3)
        parts.append(oc.reshape(NCH, ROWS_PER_CORE, OUT_W))
    full = np.concatenate(parts, axis=1)  # [256, 256, 256]
    out = full.reshape(N_B, C_CH, OUT_H, OUT_W).astype(np.float32)
    return out, res


def kernel(accumulator: np.ndarray) -> np.ndarray:
    out, _ = _run(np.asarray(accumulator, dtype=np.float32), trace=False)
    return out
